# revision 1
# baseline (speedup 1.0000x reference)
"""Distance-aware multihead attention on 8 Trainium2 NeuronCores.

Problem: B=4, S=1024, D=768, H=12, DK=64, NUM_EMB=10.
  q/k/v = linear projections of query/key/value
  idx[b,i,j] = clip(round(9 * |pos_i - pos_j| / MAXD), 0, 9)
  logits = (q.k^T + qe[b,h,i,idx[b,i,j]]) / 8   where qe = q @ emb_k^T
  out = softmax(logits) @ v

Key decompositions:
  - bias qe[...,idx] = qe[...,0] + sum_{e=1..9} (qe_e - qe_{e-1}) * (d2 >= T_e^2);
    the qe_0 term is constant along the softmax axis and cancels -> dropped.
  - step masks (d2 >= T_e^2) are shared across all 12 heads of a q-tile.
  - bias accumulated onto QK logits via 9 scalar_tensor_tensor ops per (head, q-tile).

Sharding: core c handles batch c//2, query-half c%2 (512 queries, all heads).
K/V/projections are computed per-core from full-S inputs (duplicated across the
2 cores sharing a batch); masks/logits/AV are not duplicated.

Layouts: Q^T/K^T [dim, token] f32r (from projections), V [token, dim] bf16.
P = exp((qk+bias)/8) bf16 in [q, k]; transposed to [k, q] 128-chunks via the
DMA-xbar transpose engine; AV accumulates over the 8 k-chunks on TensorE.
"""
import os
import numpy as np

import concourse.bass as bass
import concourse.tile as tile
from concourse import bacc, mybir
from concourse.bass_utils import run_bass_kernel_spmd

F32 = mybir.dt.float32
F32R = mybir.dt.float32r
BF16 = mybir.dt.bfloat16
ACT = mybir.ActivationFunctionType
ALU = mybir.AluOpType

B, S, D = 4, 1024, 768
H, DK = 12, 64
NUM_EMB = 10
MAX_DIST = 100000.0 * 2 ** 0.5
SQ = S // 2          # queries per core
NQT = SQ // 128      # q-tiles per core (4)
NKT = S // 128       # k token chunks (8)
NDT = D // 128       # dim tiles (6)
NCORES = 8

# squared thresholds: idx >= e  <=>  d2 >= ((e-0.5)*MAX_DIST/9)^2
THRESH2 = [float(((e - 0.5) * MAX_DIST / 9.0) ** 2) for e in range(1, NUM_EMB)]


def _load_T(nc, dst, src_dram, ncols):
    """src [rows, ncols*64] DRAM -> dst [128, ncols_grp, rows] = src^T, via
    64-partition xbar transpose chunks. dst is [128, n, rows] with
    dst[(64j)%128 + p64, j//2, r] = src[r, 64j + p64]."""
    for j in range(ncols // 64):
        nc.sync.dma_start_transpose(
            dst[(64 * j) % 128:(64 * j) % 128 + 64, j // 2, :],
            src_dram[:, 64 * j:64 * j + 64])


def build_nc(stage="full"):
    nc = bacc.Bacc("TRN2", target_bir_lowering=False, debug=False)

    # matmul-feeding inputs are float32r so the fp32r verifier accepts
    # DMA -> SBUF -> matmul (host values are plain fp32 bits).
    xq = nc.dram_tensor("xq", [SQ, D], F32R, kind="ExternalInput").ap()
    xk = nc.dram_tensor("xk", [S, D], F32R, kind="ExternalInput").ap()
    xv = nc.dram_tensor("xv", [S, D], F32R, kind="ExternalInput").ap()
    pos = nc.dram_tensor("pos", [S, 2], F32, kind="ExternalInput").ap()
    posq = nc.dram_tensor("posq", [SQ, 2], F32, kind="ExternalInput").ap()
    wq = nc.dram_tensor("wq", [D, D], F32R, kind="ExternalInput").ap()
    wk = nc.dram_tensor("wk", [D, D], F32R, kind="ExternalInput").ap()
    wv = nc.dram_tensor("wv", [D, D], F32R, kind="ExternalInput").ap()
    bq = nc.dram_tensor("bq", [D], F32, kind="ExternalInput").ap()
    bk = nc.dram_tensor("bk", [D], F32, kind="ExternalInput").ap()
    bv = nc.dram_tensor("bv", [D], F32, kind="ExternalInput").ap()
    emb = nc.dram_tensor("emb", [NUM_EMB, DK], F32R, kind="ExternalInput").ap()
    out = nc.dram_tensor("out", [SQ, D], F32, kind="ExternalOutput").ap()

    # debug stages: "proj" stops after projections, "masks" after d2/masks,
    # "logits" skips transpose+AV, "notrans" replaces the P transpose with a
    # plain DMA (wrong values, isolates the xbar), "full" is the real kernel.
    with tile.TileContext(nc) as tc:
        with tc.tile_pool(name="persist", bufs=1) as persist:
            # ---- setup: bias columns, position broadcasts ----
            bq_col = persist.tile([128, NDT], F32)
            bk_col = persist.tile([128, NDT], F32)
            nc.sync.dma_start(out=bq_col[:], in_=bass.AP(tensor=bq.tensor, offset=0, ap=[[1, 128], [128, NDT]]))
            nc.sync.dma_start(out=bk_col[:], in_=bass.AP(tensor=bk.tensor, offset=0, ap=[[1, 128], [128, NDT]]))
            bv_b = persist.tile([128, D], F32)
            nc.sync.dma_start(out=bv_b[:], in_=bass.AP(tensor=bv.tensor, offset=0, ap=[[0, 128], [1, D]]))
            xk_b = persist.tile([128, S], F32)
            yk_b = persist.tile([128, S], F32)
            nc.sync.dma_start(out=xk_b[:], in_=bass.AP(tensor=pos.tensor, offset=0, ap=[[0, 128], [2, S]]))
            nc.sync.dma_start(out=yk_b[:], in_=bass.AP(tensor=pos.tensor, offset=1, ap=[[0, 128], [2, S]]))
            # query positions as per-partition scalars [128, NQT]
            xq_col = persist.tile([128, NQT], F32)
            yq_col = persist.tile([128, NQT], F32)
            nc.sync.dma_start(out=xq_col[:], in_=bass.AP(tensor=posq.tensor, offset=0, ap=[[2, 128], [256, NQT]]))
            nc.sync.dma_start(out=yq_col[:], in_=bass.AP(tensor=posq.tensor, offset=1, ap=[[2, 128], [256, NQT]]))
            # emb^T on both 64-partition halves
            embT = persist.tile([128, NUM_EMB], F32R)
            nc.sync.dma_start_transpose(embT[0:64, :], emb[:, :])
            nc.sync.dma_start_transpose(embT[64:128, :], emb[:, :])
            embT_blk = persist.tile([128, 2 * NUM_EMB], F32R)
            nc.vector.memset(embT_blk[:].bitcast(F32), 0.0)
            nc.sync.dma_start_transpose(embT_blk[0:64, 0:NUM_EMB], emb[:, :])
            nc.sync.dma_start_transpose(embT_blk[64:128, NUM_EMB:2 * NUM_EMB], emb[:, :])

            ident = persist.tile([128, 128], BF16)
            from concourse.masks import make_identity
            make_identity(nc, ident[:])
            v_sb = persist.tile([128, NKT, D], BF16)   # V[token, dim], token-chunked
            kT = persist.tile([128, NDT, S], F32R)     # K^T[dim, token]
            qT = persist.tile([128, NDT, SQ], F32R)    # Q^T[dim, token]

            # ---- projections (phased so X^T/W^T buffers are freed early) ----
            with tc.tile_pool(name="vproj", bufs=1) as vp, \
                 tc.tile_pool(name="vps", bufs=2, space="PSUM") as vps:
                wvT = vp.tile([128, NDT, D], F32R)
                xvT = vp.tile([128, NDT, S], F32R)
                _load_T(nc, wvT, wv, D)
                _load_T(nc, xvT, xv, D)
                for m in range(NKT):
                    for hf in range(2):
                        ps = vps.tile([128, 384], F32, tag="pj")
                        for t in range(NDT):
                            nc.tensor.matmul(ps[:], xvT[:, t, 128 * m:128 * m + 128],
                                             wvT[:, t, 384 * hf:384 * hf + 384],
                                             start=(t == 0), stop=(t == NDT - 1))
                        nc.scalar.copy(v_sb[:, m, 384 * hf:384 * hf + 384], ps[:])

            with tc.tile_pool(name="kproj", bufs=1) as kp, \
                 tc.tile_pool(name="kps", bufs=2, space="PSUM") as kps:
                wkT = kp.tile([128, NDT, D], F32R)
                xkT = kp.tile([128, NDT, S], F32R)
                _load_T(nc, wkT, wk, D)
                _load_T(nc, xkT, xk, D)
                for m in range(NDT):
                    for hf in range(2):
                        ps = kps.tile([128, 512], F32, tag="pj")
                        for t in range(NDT):
                            nc.tensor.matmul(ps[:], wkT[:, t, 128 * m:128 * m + 128],
                                             xkT[:, t, 512 * hf:512 * hf + 512],
                                             start=(t == 0), stop=(t == NDT - 1))
                        nc.scalar.activation(kT[:, m, 512 * hf:512 * hf + 512], ps[:],
                                             ACT.Identity, bias=bk_col[:, m:m + 1])

            with tc.tile_pool(name="qproj", bufs=1) as qp, \
                 tc.tile_pool(name="qps", bufs=2, space="PSUM") as qps:
                wqT = qp.tile([128, NDT, D], F32R)
                xqT = qp.tile([128, NDT, SQ], F32R)
                _load_T(nc, wqT, wq, D)
                _load_T(nc, xqT, xq, D)
                for m in range(NDT):
                    ps = qps.tile([128, 512], F32, tag="pj")
                    for t in range(NDT):
                        nc.tensor.matmul(ps[:], wqT[:, t, 128 * m:128 * m + 128],
                                         xqT[:, t, :],
                                         start=(t == 0), stop=(t == NDT - 1))
                    nc.scalar.activation(qT[:, m, :], ps[:], ACT.Identity,
                                         bias=bq_col[:, m:m + 1])

            if stage == "proj":
                # dump some projection results and stop
                with tc.tile_pool(name="dump", bufs=1) as dp:
                    t = dp.tile([128, 512], F32)
                    nc.scalar.copy(t[:], qT[:, 0, :].bitcast(F32))
                    nc.sync.dma_start(out=out[0:128, 0:512], in_=t[:])
                    t2 = dp.tile([128, 512], F32)
                    nc.scalar.copy(t2[:], kT[:, 0, 0:512].bitcast(F32))
                    nc.sync.dma_start(out=out[128:256, 0:512], in_=t2[:])
                    t3 = dp.tile([128, 512], F32)
                    nc.vector.tensor_copy(t3[:], v_sb[:, 0, 0:512])
                    nc.sync.dma_start(out=out[256:384, 0:512], in_=t3[:])

            # ---- attention ----
            if os.environ.get("BARRIER"):
                tc.strict_bb_all_engine_barrier()
            if not os.environ.get("NOWARMXP"):
                # dummy 2-byte xbar transpose: the first 2B transpose after the
                # 4B setup transposes produces garbage (xbar mode transition);
                # this one absorbs it.
                scrap = persist.tile([128, 128], BF16)
                scrapT = persist.tile([128, 128], BF16)
                nc.vector.memset(scrap[:], 0.0)
                nc.sync.dma_start_transpose(scrapT[:], scrap[:])
            if stage != "proj":
              with tc.tile_pool(name="att", bufs=2) as att, \
                 tc.tile_pool(name="accp", bufs=2) as accp, \
                 tc.tile_pool(name="qe_ps", bufs=1, space="PSUM") as qe_ps, \
                 tc.tile_pool(name="qk_ps", bufs=2, space="PSUM") as qk_ps, \
                 tc.tile_pool(name="pt_ps", bufs=1, space="PSUM") as pt_ps, \
                 tc.tile_pool(name="av_ps", bufs=2, space="PSUM") as av_ps:
                for qt in range(1 if os.environ.get("NQT1") else (NQT if (stage not in ("masks", "logits", "d2") or os.environ.get("FULLLOOPS")) else 1)):
                    if os.environ.get("QTBARRIER"):
                        tc.strict_bb_all_engine_barrier()
                    if os.environ.get("NOMASKS"):
                        masks = att.tile([128, NUM_EMB - 1, S], BF16, tag="masks")
                        dqe = att.tile([128, H, NUM_EMB - 1], F32, tag="dqe")
                        if os.environ.get("DOD2"):
                            dx = att.tile([128, S], F32, tag="dx")
                            dy = att.tile([128, S], F32, tag="dy")
                            nc.vector.tensor_scalar(out=dx[:], in0=xk_b[:], scalar1=xq_col[:, qt:qt + 1],
                                                    scalar2=None, op0=ALU.subtract)
                            nc.vector.tensor_scalar(out=dy[:], in0=yk_b[:], scalar1=yq_col[:, qt:qt + 1],
                                                    scalar2=None, op0=ALU.subtract)
                            dx2 = att.tile([128, S], F32, tag="dx2")
                            dy2 = att.tile([128, S], F32, tag="dy2")
                            nc.scalar.square(dx2[:], dx[:])
                            nc.scalar.square(dy2[:], dy[:])
                            d2 = att.tile([128, S], F32, tag="d2")
                            nc.vector.tensor_add(d2[:], dx2[:], dy2[:])
                            if os.environ.get("DOMASKS"):
                                for e in range(NUM_EMB - 1):
                                    nc.vector.tensor_scalar(out=masks[:, e, :], in0=d2[:],
                                                            scalar1=THRESH2[e], scalar2=None,
                                                            op0=ALU.is_ge)
                        if os.environ.get("SECTBARRIER"):
                            tc.strict_bb_all_engine_barrier()
                        if os.environ.get("DOQE"):
                            qe_psum = qe_ps.tile([128, H * NUM_EMB], F32, tag="qe")
                            if os.environ.get("QEBLK"):
                                for m in range(NDT):
                                    nc.tensor.matmul(qe_psum[:, 20 * m:20 * m + 20],
                                                     qT[:, m, 128 * qt:128 * qt + 128],
                                                     embT_blk[:],
                                                     start=True, stop=True)
                            else:
                                for h in range(H):
                                    off = (64 * h) % 128
                                    nc.tensor.matmul(qe_psum[:, 10 * h:10 * h + 10],
                                                     qT[off:off + 64, h // 2, 128 * qt:128 * qt + 128],
                                                     embT[off:off + 64, :],
                                                     start=True, stop=True)
                            qe_sb = att.tile([128, H, NUM_EMB], F32, tag="qe_sb")
                            nc.scalar.copy(qe_sb[:], qe_psum[:].rearrange("p (h e) -> p h e", e=NUM_EMB))
                            nc.vector.tensor_tensor(out=dqe[:], in0=qe_sb[:, :, 1:],
                                                    in1=qe_sb[:, :, :-1], op=ALU.subtract)
                        if os.environ.get("SECTBARRIER"):
                            tc.strict_bb_all_engine_barrier()
                        for h in range(H):
                            off = 0 if os.environ.get("OFF0") else (64 * h) % 128
                            qk = qk_ps.tile([128, S], F32, tag="qk")
                            for hf in range(2):
                                nc.tensor.matmul(qk[:, 512 * hf:512 * hf + 512],
                                                 qT[off:off + 64, h // 2, 128 * qt:128 * qt + 128],
                                                 kT[off:off + 64, h // 2, 512 * hf:512 * hf + 512],
                                                 start=True, stop=True)
                            o3 = att.tile([128, DK], F32, tag="o")
                            nc.scalar.copy(o3[:], qk[:, 0:DK])
                            nc.sync.dma_start(out=out[128 * qt:128 * qt + 128, 64 * h:64 * h + 64],
                                              in_=o3[:])
                        continue
                    # --- d2 for this q-tile: [128, S] fp32 ---
                    dx = att.tile([128, S], F32, tag="dx")
                    dy = att.tile([128, S], F32, tag="dy")
                    nc.vector.tensor_scalar(out=dx[:], in0=xk_b[:], scalar1=xq_col[:, qt:qt + 1],
                                            scalar2=None, op0=ALU.subtract)
                    nc.vector.tensor_scalar(out=dy[:], in0=yk_b[:], scalar1=yq_col[:, qt:qt + 1],
                                            scalar2=None, op0=ALU.subtract)
                    dx2 = att.tile([128, S], F32, tag="dx2")
                    dy2 = att.tile([128, S], F32, tag="dy2")
                    nc.scalar.square(dx2[:], dx[:])
                    nc.scalar.square(dy2[:], dy[:])
                    d2 = att.tile([128, S], F32, tag="d2")
                    nc.vector.tensor_add(d2[:], dx2[:], dy2[:])

                    if stage == "qeonly":
                        qe_psum = qe_ps.tile([128, H * NUM_EMB], F32, tag="qe")
                        for h in range(H):
                            off = (64 * h) % 128
                            nc.tensor.matmul(qe_psum[:, 10 * h:10 * h + 10],
                                             qT[off:off + 64, h // 2, 128 * qt:128 * qt + 128],
                                             embT[off:off + 64, :],
                                             start=True, stop=True)
                        qe_sb = att.tile([128, H, NUM_EMB], F32, tag="qe_sb")
                        nc.scalar.copy(qe_sb[:], qe_psum[:].rearrange("p (h e) -> p h e", e=NUM_EMB))
                        dqe = att.tile([128, H, NUM_EMB - 1], F32, tag="dqe")
                        nc.vector.tensor_tensor(out=dqe[:], in0=qe_sb[:, :, 1:],
                                                in1=qe_sb[:, :, :-1], op=ALU.subtract)
                        o4 = att.tile([128, DK], F32, tag="o")
                        nc.vector.tensor_copy(o4[:, 0:63], dqe[:, 0:7, 0:9].rearrange("p a b -> p (a b)"))
                        nc.vector.tensor_copy(o4[:, 63:64], dqe[:, 7, 0:1])
                        nc.sync.dma_start(out=out[128 * qt:128 * qt + 128, 0:DK], in_=o4[:])
                        continue

                    if stage == "d2":
                        nc.sync.dma_start(out=out[128:256, 0:D], in_=d2[:, 0:D])
                        continue

                    # --- step masks [128, 9, S] bf16 ---
                    nmask = int(os.environ.get("NMASKS", str(NUM_EMB - 1)))
                    mdt = F32 if os.environ.get("MASKF32") else BF16
                    masks = att.tile([128, NUM_EMB - 1, S], mdt, tag="masks")
                    for e in range(nmask):
                        if os.environ.get("MASKCOPY"):
                            nc.vector.tensor_copy(masks[:, e, :], d2[:])
                        elif os.environ.get("MASKIMM1"):
                            nc.vector.tensor_scalar(out=masks[:, e, :], in0=d2[:],
                                                    scalar1=1.0, scalar2=None,
                                                    op0=ALU.is_ge)
                        else:
                            nc.vector.tensor_scalar(out=masks[:, e, :], in0=d2[:],
                                                    scalar1=THRESH2[e], scalar2=None,
                                                    op0=ALU.is_ge)

                    # --- qe -> dqe for this q-tile (block-diagonal: 2 heads per matmul;
                    # 64-partition sliver matmuls into one bank proved flaky on HW) ---
                    qe_psum = qe_ps.tile([128, H * NUM_EMB], F32, tag="qe")
                    for m in range(NDT):
                        nc.tensor.matmul(qe_psum[:, 20 * m:20 * m + 20],
                                         qT[:, m, 128 * qt:128 * qt + 128],
                                         embT_blk[:],
                                         start=True, stop=True)
                    qe_sb = att.tile([128, H, NUM_EMB], F32, tag="qe_sb")
                    nc.scalar.copy(qe_sb[:], qe_psum[:].rearrange("p (h e) -> p h e", e=NUM_EMB))
                    dqe = att.tile([128, H, NUM_EMB - 1], F32, tag="dqe")
                    nc.vector.tensor_tensor(out=dqe[:], in0=qe_sb[:, :, 1:],
                                            in1=qe_sb[:, :, :-1], op=ALU.subtract)

                    if stage == "masks":
                        if not os.environ.get("NODUMP"):
                            md = att.tile([128, S], F32, tag="md")
                            nc.vector.tensor_copy(md[:], masks[:, 0, :])
                            nc.sync.dma_start(out=out[0:128, 0:D], in_=md[:, 0:D])
                        nc.sync.dma_start(out=out[128:256, 0:D], in_=d2[:, 0:D])
                        continue

                    for h in range(H if (stage != "logits" or os.environ.get("FULLLOOPS")) else 1):
                        off = 0 if os.environ.get("OFF0") else (64 * h) % 128
                        # --- logits = q.k^T ---
                        qk = qk_ps.tile([128, S], F32, tag="qk")
                        for hf in range(2):
                            nc.tensor.matmul(qk[:, 512 * hf:512 * hf + 512],
                                             qT[off:off + 64, h // 2, 128 * qt:128 * qt + 128],
                                             kT[off:off + 64, h // 2, 512 * hf:512 * hf + 512],
                                             start=True, stop=True)
                        # --- + bias: 9 chained masked MACs ---
                        src = qk
                        if stage == "qkonly":
                            o3 = att.tile([128, DK], F32, tag="o")
                            nc.scalar.copy(o3[:], qk[:, 0:DK])
                            nc.sync.dma_start(out=out[128 * qt:128 * qt + 128, 64 * h:64 * h + 64],
                                              in_=o3[:])
                            continue
        
                        nstt = 0 if stage == "qkexp" else (NUM_EMB - 1)
                        for e in range(nstt):
                            acc = accp.tile([128, S], F32, tag="acc")
                            nc.vector.scalar_tensor_tensor(
                                out=acc[:], in0=masks[:, e, :], scalar=dqe[:, h, e:e + 1],
                                in1=src[:], op0=ALU.mult, op1=ALU.add)
                            src = acc
                        if stage == "sttonly":
                            o3 = att.tile([128, DK], F32, tag="o")
                            nc.vector.tensor_copy(o3[:], src[:, 0:DK])
                            nc.sync.dma_start(out=out[128 * qt:128 * qt + 128, 64 * h:64 * h + 64],
                                              in_=o3[:])
                            continue
                        # --- P = exp(logits/8), row-sum, transpose ---
                        p_sb = att.tile([128, S], BF16, tag="p")
                        den = att.tile([128, 1], F32, tag="den")
                        nc.scalar.activation(p_sb[:], src[:], ACT.Exp, scale=0.125,
                                             accum_out=den[:])
                        if stage in ("logits", "qkexp"):
                            pf = att.tile([128, S], F32, tag="pf")
                            nc.vector.tensor_copy(pf[:], p_sb[:])
                            nc.sync.dma_start(out=out[0:128, 0:D], in_=pf[:, 0:D])
                            continue
                        if os.environ.get("PSTAGE"):
                            p2 = att.tile([128, S], BF16, tag="p2")
                            nc.vector.tensor_copy(p2[:], p_sb[:])
                            p_sb = p2
                        pT = att.tile([128, NKT, 128], BF16, tag="pT")
                        if stage in ("notrans", "nopt", "av"):
                            nc.sync.dma_start(out=pT[:], in_=p_sb[:].rearrange("p (c j) -> p c j", j=128))
                        elif os.environ.get("XBARTRANS"):
                            # xbar transpose is only correct up to 512-wide inputs;
                            # first-op-in-kernel also glitches (see PE path below)
                            nc.sync.dma_start_transpose(pT[:, 0:NKT // 2, :], p_sb[:, 0:S // 2])
                            nc.sync.dma_start_transpose(pT[:, NKT // 2:NKT, :], p_sb[:, S // 2:S])
                        else:
                            ptp = pt_ps.tile([128, NKT, 128], BF16, tag="ptp")
                            for c in range(NKT):
                                nc.tensor.transpose(ptp[:, c, :], p_sb[:, 128 * c:128 * c + 128], ident[:])
                            nc.scalar.copy(pT[:], ptp[:])
                        # --- out_h = (P^T . V_h) / den + bv_h ---
                        if stage == "nopt":
                            # skip everything after exp except a pT dump
                            o2 = att.tile([128, DK], F32, tag="o")
                            nc.vector.tensor_copy(o2[:], pT[:, 0, 0:DK])
                            nc.sync.dma_start(out=out[128 * qt:128 * qt + 128, 64 * h:64 * h + 64],
                                              in_=o2[:])
                            continue
                        if os.environ.get("PTCOPY"):
                            pT2 = att.tile([128, NKT, 128], BF16, tag="pT2")
                            nc.vector.tensor_copy(pT2[:], pT[:])
                            pT = pT2
                        av = av_ps.tile([128, DK], F32, tag="av")
                        for c in range(NKT):
                            nc.tensor.matmul(av[:], pT[:, c, :], v_sb[:, c, 64 * h:64 * h + 64],
                                             start=(c == 0), stop=(c == NKT - 1))
                        if stage == "av":
                            o2 = att.tile([128, DK], F32, tag="o")
                            nc.scalar.copy(o2[:], av[:])
                            nc.sync.dma_start(out=out[128 * qt:128 * qt + 128, 64 * h:64 * h + 64],
                                              in_=o2[:])
                            continue
                        recip = att.tile([128, 1], F32, tag="recip")
                        nc.vector.reciprocal(recip[:], den[:])
                        o_sb = att.tile([128, DK], F32, tag="o")
                        nc.vector.scalar_tensor_tensor(
                            out=o_sb[:], in0=av[:], scalar=recip[:],
                            in1=bv_b[:, 64 * h:64 * h + 64], op0=ALU.mult, op1=ALU.add)
                        nc.sync.dma_start(out=out[128 * qt:128 * qt + 128, 64 * h:64 * h + 64],
                                          in_=o_sb[:])
    nc.compile()
    return nc


_NC_CACHE = {}


def _get_nc():
    if "nc" not in _NC_CACHE:
        _NC_CACHE["nc"] = build_nc()
    return _NC_CACHE["nc"]


def kernel(query, key, value, tile_positions, Wq, bq, Wk, bk, Wv, bv, emb_k):
    query = np.ascontiguousarray(np.asarray(query, dtype=np.float32))
    key = np.ascontiguousarray(np.asarray(key, dtype=np.float32))
    value = np.ascontiguousarray(np.asarray(value, dtype=np.float32))
    tile_positions = np.ascontiguousarray(np.asarray(tile_positions, dtype=np.float32))
    Wq = np.ascontiguousarray(np.asarray(Wq, dtype=np.float32))
    Wk = np.ascontiguousarray(np.asarray(Wk, dtype=np.float32))
    Wv = np.ascontiguousarray(np.asarray(Wv, dtype=np.float32))
    bq = np.ascontiguousarray(np.asarray(bq, dtype=np.float32))
    bk = np.ascontiguousarray(np.asarray(bk, dtype=np.float32))
    bv = np.ascontiguousarray(np.asarray(bv, dtype=np.float32))
    emb_k = np.ascontiguousarray(np.asarray(emb_k, dtype=np.float32))

    nc = _get_nc()
    in_maps = []
    for c in range(NCORES):
        b, qh = c // 2, c % 2
        in_maps.append({
            "xq": np.ascontiguousarray(query[b, qh * SQ:(qh + 1) * SQ]),
            "xk": key[b], "xv": value[b],
            "pos": tile_positions[b],
            "posq": np.ascontiguousarray(tile_positions[b, qh * SQ:(qh + 1) * SQ]),
            "wq": Wq, "wk": Wk, "wv": Wv,
            "bq": bq, "bk": bk, "bv": bv,
            "emb": emb_k,
        })
    res = run_bass_kernel_spmd(nc, in_maps, core_ids=list(range(NCORES)))
    out = np.empty((B, S, D), np.float32)
    for c in range(NCORES):
        b, qh = c // 2, c % 2
        out[b, qh * SQ:(qh + 1) * SQ] = res.results[c]["out"]
    return out



# revision 4
# speedup vs baseline: 16104.5099x; 16104.5099x over previous
"""Distance-aware multihead attention on 8 Trainium2 NeuronCores.

Problem: B=4, S=1024, D=768, H=12, DK=64, NUM_EMB=10.
  q/k/v = linear projections of query/key/value
  idx[b,i,j] = clip(round(9 * |pos_i - pos_j| / MAXD), 0, 9)
  logits = (q.k^T + qe[b,h,i,idx[b,i,j]]) / 8   where qe = q @ emb_k^T
  out = softmax(logits) @ v

Design (measured 364us on HW vs 3453us for the v1 baseline):
  - All transposed operands (W^T, X^T, emb^T, pos^T) are prepared host-side
    as contiguous arrays -> plain strided DMA loads (v1 spent 2.6ms of its
    3.45ms in serialized DMA-xbar transposes of W/X).
  - Key-position row broadcasts ([128,S]) via stride-0 DMA from contiguous
    posT rows (1 descriptor/partition; exact, unlike a f32r ones-matmul).
  - Distance bias: qe[...,idx] = qe_0 (constant along the softmax axis,
    cancels -> dropped) + sum_e dqe_e * (d2 >= T_e^2). All nine step terms
    are accumulated INTO THE QK PSUM by the tensor engine (matmuls with
    start=False), so the vector engine never runs wide adds:
      * steps 0-1: Act scaled-masks (Identity, scale=dqe), pair-merged on
        DVE, folded via an identity matmul;
      * steps 2-3: DVE fused tensor_scalar (d2>=T, *dqe), pair-merged,
        identity-folded;
      * steps 4-8: PE diag-folds — qk += diag(dqe_e) @ mask_e, where the
        [128,128] diag tile is built on DVE as ident*dqe (tiny op) and
        mask_e is the per-q-tile threshold mask (head-independent).
    Everything fp16 except d2/PSUM/den (f32). gpsimd is unusable for wide
    tensor ops on HW (~16us per [128,1024] op + SBUF-port contention).
  - exp(logits/8) on Act reads the PSUM directly, accum_out -> softmax den.
  - P transposed on PE (identity matmul) + DVE copy; AV + den-recip scale.
  - Per-pair emission is software-pipelined (exp/AV block of pair i emitted
    after the SM/fold block of pair i+1); outputs batched per q-tile.

Sharding: core c handles batch c//2, query-half c%2 (512 queries, all heads).
"""
import os
import numpy as np

import concourse.bass as bass
import concourse.tile as tile
from concourse import bacc, mybir
from concourse.bass_utils import run_bass_kernel_spmd
from concourse.masks import make_identity

F32 = mybir.dt.float32
F32R = mybir.dt.float32r
BF16 = mybir.dt.float16  # fp16: 3 more mantissa bits than bf16, same DVE 2x speed
ACT = mybir.ActivationFunctionType
ALU = mybir.AluOpType

B, S, D = 4, 1024, 768
H, DK = 12, 64
NUM_EMB = 10
MAX_DIST = 100000.0 * 2 ** 0.5
SQ = S // 2          # queries per core
NQT = SQ // 128      # q-tiles per core (4)
NKT = S // 128       # k token chunks (8)
NDT = D // 128       # dim tiles (6)
NCORES = 8

# squared thresholds: idx >= e+1  <=>  d2 >= ((e+0.5)*MAX_DIST/9)^2
THRESH2 = [float(((e - 0.5) * MAX_DIST / 9.0) ** 2) for e in range(1, NUM_EMB)]

# step -> engine assignment (9 steps, indices 0..8 into THRESH2/dqe).
# All scaled-mask (SM) tiles are summed into the QK PSUM via PE identity-
# matmul accumulation (start=False); no vector-engine adds.
ACT_STEPS = [0, 1]          # Act scaled-mask (Identity, scale=dqe)
DVE_STEPS = [2, 3]          # DVE fused (d2 >= T)*dqe tensor_scalar
DIAG_STEPS = [4, 5, 6, 7, 8]  # PE diag(dqe)-matmul folds of the raw mask
MASK_STEPS = list(range(9))  # all 9 masks materialized per q-tile
# (gpsimd is useless for tensor ops on HW: ~16us per [128,1024] tensor_scalar
# and its shared SBUF port starves the DVE)


def build_nc():
    nc = bacc.Bacc("TRN2", target_bir_lowering=False, debug=False)

    xqT = nc.dram_tensor("xqT", [D, SQ], F32R, kind="ExternalInput").ap()
    xkT = nc.dram_tensor("xkT", [D, S], F32R, kind="ExternalInput").ap()
    xvT = nc.dram_tensor("xvT", [D, S], F32R, kind="ExternalInput").ap()
    wqT = nc.dram_tensor("wqT", [D, D], F32R, kind="ExternalInput").ap()
    wkT = nc.dram_tensor("wkT", [D, D], F32R, kind="ExternalInput").ap()
    wvT = nc.dram_tensor("wvT", [D, D], F32R, kind="ExternalInput").ap()
    bq = nc.dram_tensor("bq", [D], F32, kind="ExternalInput").ap()
    bk = nc.dram_tensor("bk", [D], F32, kind="ExternalInput").ap()
    bv = nc.dram_tensor("bv", [D], F32, kind="ExternalInput").ap()
    posT = nc.dram_tensor("posT", [2, S], F32, kind="ExternalInput").ap()
    posqT = nc.dram_tensor("posqT", [2, SQ], F32, kind="ExternalInput").ap()
    embT = nc.dram_tensor("embT", [DK, NUM_EMB], BF16, kind="ExternalInput").ap()
    out = nc.dram_tensor("out", [SQ, D], F32, kind="ExternalOutput").ap()

    with tile.TileContext(nc) as tc:
        with tc.tile_pool(name="persist", bufs=1) as persist:
            # ---- small setup: bias columns, query-position scalars ----
            bq_col = persist.tile([128, NDT], F32)
            bk_col = persist.tile([128, NDT], F32)
            nc.sync.dma_start(out=bq_col[:], in_=bass.AP(tensor=bq.tensor, offset=0, ap=[[1, 128], [128, NDT]]))
            nc.sync.dma_start(out=bk_col[:], in_=bass.AP(tensor=bk.tensor, offset=0, ap=[[1, 128], [128, NDT]]))
            bv_b = persist.tile([128, D], F32)
            nc.sync.dma_start(out=bv_b[:], in_=bass.AP(tensor=bv.tensor, offset=0, ap=[[0, 128], [1, D]]))
            xq_col = persist.tile([128, NQT], F32)
            yq_col = persist.tile([128, NQT], F32)
            nc.sync.dma_start(out=xq_col[:], in_=bass.AP(tensor=posqT.tensor, offset=0, ap=[[1, 128], [128, NQT]]))
            nc.sync.dma_start(out=yq_col[:], in_=bass.AP(tensor=posqT.tensor, offset=SQ, ap=[[1, 128], [128, NQT]]))
            # emb^T on both 64-partition halves + block-diagonal (2 heads/matmul)
            embT_sb = persist.tile([128, NUM_EMB], BF16)
            nc.sync.dma_start(out=embT_sb[0:64, :], in_=embT[:, :])
            nc.sync.dma_start(out=embT_sb[64:128, :], in_=embT[:, :])
            embT_blk = persist.tile([128, 2 * NUM_EMB], BF16)
            nc.vector.memset(embT_blk[:], 0.0)
            nc.sync.dma_start(out=embT_blk[0:64, 0:NUM_EMB], in_=embT[:, :])
            nc.sync.dma_start(out=embT_blk[64:128, NUM_EMB:2 * NUM_EMB], in_=embT[:, :])

            ident = persist.tile([128, 128], BF16)
            make_identity(nc, ident[:])

            # ---- key-position rows broadcast to all partitions via PE ----
            # broadcast the contiguous posT rows to all 128 partitions via
            # stride-0 DMA (1 contiguous descriptor per partition — cheap and
            # exact; a PE ones-matmul broadcast loses ~1.6e-4 to f32r limbs)
            xk_b = persist.tile([128, S], F32)
            yk_b = persist.tile([128, S], F32)
            nc.sync.dma_start(out=xk_b[:], in_=bass.AP(tensor=posT.tensor, offset=0, ap=[[0, 128], [1, S]]))
            nc.sync.dma_start(out=yk_b[:], in_=bass.AP(tensor=posT.tensor, offset=S, ap=[[0, 128], [1, S]]))

            # ---- persistent projection outputs ----
            v_sb = persist.tile([128, NKT, D], BF16)   # V[token, dim], token-chunked
            kT = persist.tile([128, NDT, S], BF16)     # K^T[dim, token] fp16
            qT = persist.tile([128, NDT, SQ], BF16)    # Q^T[dim, token] fp16

            # ---- projections (phased so X^T/W^T buffers are freed early) ----
            def load_T(dst, src, ncols):
                # src DRAM [D, ncols] -> dst [128, NDT, ncols]
                nc.sync.dma_start(out=dst[:], in_=bass.AP(
                    tensor=src.tensor, offset=0,
                    ap=[[ncols, 128], [128 * ncols, NDT], [1, ncols]]))

            with tc.tile_pool(name="vproj", bufs=1) as vp, \
                 tc.tile_pool(name="vps", bufs=2, space="PSUM") as vps:
                wvT_sb = vp.tile([128, NDT, D], F32R)
                xvT_sb = vp.tile([128, NDT, S], F32R)
                load_T(wvT_sb, wvT, D)
                load_T(xvT_sb, xvT, S)
                for m in range(NKT):
                    for hf in range(2):
                        ps = vps.tile([128, 384], F32, tag="pj")
                        for t in range(NDT):
                            nc.tensor.matmul(ps[:], xvT_sb[:, t, 128 * m:128 * m + 128],
                                             wvT_sb[:, t, 384 * hf:384 * hf + 384],
                                             start=(t == 0), stop=(t == NDT - 1))
                        nc.scalar.copy(v_sb[:, m, 384 * hf:384 * hf + 384], ps[:])

            with tc.tile_pool(name="kproj", bufs=1) as kp, \
                 tc.tile_pool(name="kps", bufs=2, space="PSUM") as kps:
                wkT_sb = kp.tile([128, NDT, D], F32R)
                xkT_sb = kp.tile([128, NDT, S], F32R)
                load_T(wkT_sb, wkT, D)
                load_T(xkT_sb, xkT, S)
                for m in range(NDT):
                    for hf in range(2):
                        ps = kps.tile([128, 512], F32, tag="pj")
                        for t in range(NDT):
                            nc.tensor.matmul(ps[:], wkT_sb[:, t, 128 * m:128 * m + 128],
                                             xkT_sb[:, t, 512 * hf:512 * hf + 512],
                                             start=(t == 0), stop=(t == NDT - 1))
                        nc.scalar.activation(kT[:, m, 512 * hf:512 * hf + 512], ps[:],
                                             ACT.Identity, bias=bk_col[:, m:m + 1])

            with tc.tile_pool(name="qproj", bufs=1) as qp, \
                 tc.tile_pool(name="qps", bufs=2, space="PSUM") as qps:
                wqT_sb = qp.tile([128, NDT, D], F32R)
                xqT_sb = qp.tile([128, NDT, SQ], F32R)
                load_T(wqT_sb, wqT, D)
                load_T(xqT_sb, xqT, SQ)
                for m in range(NDT):
                    ps = qps.tile([128, 512], F32, tag="pj")
                    for t in range(NDT):
                        nc.tensor.matmul(ps[:], wqT_sb[:, t, 128 * m:128 * m + 128],
                                         xqT_sb[:, t, :],
                                         start=(t == 0), stop=(t == NDT - 1))
                    nc.scalar.activation(qT[:, m, :], ps[:], ACT.Identity,
                                         bias=bq_col[:, m:m + 1])

            # ---- qe -> dqe for all q-tiles (needs qT) ----
            dqe_all = persist.tile([128, NQT, H, NUM_EMB - 1], F32)
            with tc.tile_pool(name="qe_sbp", bufs=2) as qe_sbp, \
                 tc.tile_pool(name="qe_ps", bufs=2, space="PSUM") as qe_ps:
                for qt in range(NQT):
                    qe_psum = qe_ps.tile([128, H * NUM_EMB], F32, tag="qe")
                    for m in range(NDT):
                        nc.tensor.matmul(qe_psum[:, 20 * m:20 * m + 20],
                                         qT[:, m, 128 * qt:128 * qt + 128],
                                         embT_blk[:],
                                         start=True, stop=True)
                    qe_sb = qe_sbp.tile([128, H, NUM_EMB], F32, tag="qe_sb")
                    nc.scalar.copy(qe_sb[:], qe_psum[:].rearrange("p (h e) -> p h e", e=NUM_EMB))
                    nc.vector.tensor_tensor(out=dqe_all[:, qt, :, :], in0=qe_sb[:, :, 1:],
                                            in1=qe_sb[:, :, :-1], op=ALU.subtract)

            # ---- attention ----
            with tc.tile_pool(name="sm", bufs=2) as smp, \
                 tc.tile_pool(name="att", bufs=2) as att, \
                 tc.tile_pool(name="prep", bufs=2) as prep, \
                 tc.tile_pool(name="oq", bufs=2) as oqp, \
                 tc.tile_pool(name="qk_ps", bufs=3, space="PSUM") as qk_ps, \
                 tc.tile_pool(name="pt_ps", bufs=1, space="PSUM") as pt_ps, \
                 tc.tile_pool(name="av_ps", bufs=1, space="PSUM") as av_ps:
                d2_of = {}
                masks_of = {}

                def emit_prep(qt):
                    # per-qt prep: d2 + masks (depends only on positions)
                    dx = att.tile([128, S], F32, tag="dx")
                    dy = att.tile([128, S], F32, tag="dy")
                    nc.vector.tensor_scalar(out=dx[:], in0=xk_b[:], scalar1=xq_col[:, qt:qt + 1],
                                            scalar2=None, op0=ALU.subtract)
                    nc.vector.tensor_scalar(out=dy[:], in0=yk_b[:], scalar1=yq_col[:, qt:qt + 1],
                                            scalar2=None, op0=ALU.subtract)
                    dx2 = att.tile([128, S], F32, tag="dx2")
                    dy2 = att.tile([128, S], F32, tag="dy2")
                    nc.scalar.square(dx2[:], dx[:])
                    nc.scalar.square(dy2[:], dy[:])
                    d2t = prep.tile([128, S], F32, tag="d2")
                    nc.vector.tensor_add(d2t[:], dx2[:], dy2[:])
                    maskst = prep.tile([128, len(MASK_STEPS), S], BF16, tag="masks")
                    for i, e in enumerate(MASK_STEPS):
                        nc.vector.tensor_scalar(out=maskst[:, i, :], in0=d2t[:],
                                                scalar1=THRESH2[e], scalar2=None,
                                                op0=ALU.is_ge)
                    d2_of[qt] = d2t
                    masks_of[qt] = maskst

                def emit_front(qt, h):
                    # SM generation + QK/fold accumulation into PSUM
                    d2 = d2_of[qt][:]
                    masks = masks_of[qt]
                    dqe = dqe_all[:, qt, h, :]
                    sm_a = []
                    for i, e in enumerate(ACT_STEPS):
                        sm = smp.tile([128, S], BF16, tag=f"sma{i}")
                        nc.scalar.activation(sm[:], masks[:, i, :], ACT.Identity,
                                             scale=dqe[:, e:e + 1])
                        sm_a.append(sm)
                    sm_v = []
                    for i, e in enumerate(DVE_STEPS):
                        sm = smp.tile([128, S], BF16, tag=f"smv{i}")
                        nc.vector.tensor_scalar(out=sm[:], in0=d2,
                                                scalar1=THRESH2[e], scalar2=dqe[:, e:e + 1],
                                                op0=ALU.is_ge, op1=ALU.mult)
                        sm_v.append(sm)
                    # DVE pre-merges: two SM pairs -> two identity folds
                    m1 = att.tile([128, S], BF16, tag="m1")
                    nc.vector.tensor_add(m1[:], sm_a[0][:], sm_a[1][:])
                    m2 = att.tile([128, S], BF16, tag="m2")
                    nc.vector.tensor_add(m2[:], sm_v[0][:], sm_v[1][:])
                    # diag(dqe_e) tiles for the PE diag-folds (tiny DVE ops)
                    diags = []
                    for i, e in enumerate(DIAG_STEPS):
                        dg = smp.tile([128, 128], BF16, tag=f"diag{i}")
                        nc.vector.tensor_scalar(out=dg[:], in0=ident[:],
                                                scalar1=dqe[:, e:e + 1], scalar2=None,
                                                op0=ALU.mult)
                        diags.append(dg)

                    qk = qk_ps.tile([128, S], F32, tag="qk")
                    # first identity fold opens the accumulation (start=True)
                    for hf in range(2):
                        nc.tensor.matmul(qk[:, 512 * hf:512 * hf + 512],
                                         ident[:], m1[:, 512 * hf:512 * hf + 512],
                                         start=True, stop=False, skip_group_check=True)
                    # logits accumulate on top: q.k^T (fp16 operands)
                    off = (64 * h) % 128
                    for hf in range(2):
                        nc.tensor.matmul(qk[:, 512 * hf:512 * hf + 512],
                                         qT[off:off + 64, h // 2, 128 * qt:128 * qt + 128],
                                         kT[off:off + 64, h // 2, 512 * hf:512 * hf + 512],
                                         start=False, stop=False, skip_group_check=True)
                    # second identity fold
                    for hf in range(2):
                        nc.tensor.matmul(qk[:, 512 * hf:512 * hf + 512],
                                         ident[:], m2[:, 512 * hf:512 * hf + 512],
                                         start=False, stop=False, skip_group_check=True)
                    # diag folds: qk += diag(dqe_e) @ mask_e
                    for i, e in enumerate(DIAG_STEPS):
                        last = i == len(DIAG_STEPS) - 1
                        for hf in range(2):
                            nc.tensor.matmul(qk[:, 512 * hf:512 * hf + 512],
                                             diags[i][:], masks[:, e, 512 * hf:512 * hf + 512],
                                             start=False, stop=last,
                                             skip_group_check=True)
                    return qk

                oq_of = {}

                def emit_back(qt, h, qk):
                    # exp, P^T, AV, output
                    p_sb = att.tile([128, S], BF16, tag="p")
                    den = att.tile([128, 1], F32, tag="den")
                    nc.scalar.activation(p_sb[:], qk[:], ACT.Exp, scale=0.125,
                                         accum_out=den[:])
                    ptp = pt_ps.tile([128, NKT, 128], BF16, tag="ptp")
                    for c in range(NKT):
                        nc.tensor.transpose(ptp[:, c, :], p_sb[:, 128 * c:128 * c + 128], ident[:])
                    pT = att.tile([128, NKT, 128], BF16, tag="pT")
                    nc.vector.tensor_copy(pT[:], ptp[:])
                    av = av_ps.tile([128, DK], F32, tag="av")
                    for c in range(NKT):
                        nc.tensor.matmul(av[:], pT[:, c, :], v_sb[:, c, 64 * h:64 * h + 64],
                                         start=(c == 0), stop=(c == NKT - 1))
                    recip = att.tile([128, 1], F32, tag="recip")
                    nc.vector.reciprocal(recip[:], den[:])
                    if h == 0:
                        o_qt_new = oqp.tile([128, D], F32, tag="oq")
                        oq_of[qt] = o_qt_new
                    o_qt = oq_of[qt]
                    nc.vector.scalar_tensor_tensor(
                        out=o_qt[:, 64 * h:64 * h + 64], in0=av[:], scalar=recip[:],
                        in1=bv_b[:, 64 * h:64 * h + 64], op0=ALU.mult, op1=ALU.add)
                    if h == H - 1:
                        nc.sync.dma_start(out=out[128 * qt:128 * qt + 128, :], in_=o_qt[:])

                # software pipeline: pair i's exp/AV block is emitted after
                # pair i+1's SM/fold block so the in-order Act queue never
                # head-of-line blocks on the PE fold chain.
                pairs = [(qt, h) for qt in range(NQT) for h in range(H)]
                pending = None
                for qt, h in pairs:
                    if h == 0:
                        emit_prep(qt)
                    qk = emit_front(qt, h)
                    if pending is not None:
                        emit_back(*pending)
                    pending = (qt, h, qk)
                emit_back(*pending)
    nc.compile()
    return nc


_NC_CACHE = {}


def _get_nc():
    if "nc" not in _NC_CACHE:
        _NC_CACHE["nc"] = build_nc()
    return _NC_CACHE["nc"]


def _make_in_maps(inputs):
    query = np.ascontiguousarray(np.asarray(inputs["query"], dtype=np.float32))
    key = np.ascontiguousarray(np.asarray(inputs["key"], dtype=np.float32))
    value = np.ascontiguousarray(np.asarray(inputs["value"], dtype=np.float32))
    tp = np.ascontiguousarray(np.asarray(inputs["tile_positions"], dtype=np.float32))
    WqT = np.ascontiguousarray(np.asarray(inputs["Wq"], dtype=np.float32).T)
    WkT = np.ascontiguousarray(np.asarray(inputs["Wk"], dtype=np.float32).T)
    WvT = np.ascontiguousarray(np.asarray(inputs["Wv"], dtype=np.float32).T)
    bq = np.ascontiguousarray(np.asarray(inputs["bq"], dtype=np.float32))
    bk = np.ascontiguousarray(np.asarray(inputs["bk"], dtype=np.float32))
    bv = np.ascontiguousarray(np.asarray(inputs["bv"], dtype=np.float32))
    embT = np.ascontiguousarray(np.asarray(inputs["emb_k"], dtype=np.float32).T.astype(np.float16))

    xkT = [np.ascontiguousarray(key[b].T) for b in range(B)]
    xvT = [np.ascontiguousarray(value[b].T) for b in range(B)]
    xqTf = [np.ascontiguousarray(query[b].T) for b in range(B)]
    posT = [np.ascontiguousarray(tp[b].T) for b in range(B)]

    in_maps = []
    for c in range(NCORES):
        b, qh = c // 2, c % 2
        in_maps.append({
            "xqT": np.ascontiguousarray(xqTf[b][:, qh * SQ:(qh + 1) * SQ]),
            "xkT": xkT[b], "xvT": xvT[b],
            "posT": posT[b],
            "posqT": np.ascontiguousarray(posT[b][:, qh * SQ:(qh + 1) * SQ]),
            "wqT": WqT, "wkT": WkT, "wvT": WvT,
            "bq": bq, "bk": bk, "bv": bv,
            "embT": embT,
        })
    return in_maps


def kernel(query, key, value, tile_positions, Wq, bq, Wk, bk, Wv, bv, emb_k):
    inputs = {"query": query, "key": key, "value": value,
              "tile_positions": tile_positions,
              "Wq": Wq, "bq": bq, "Wk": Wk, "bk": bk, "Wv": Wv, "bv": bv,
              "emb_k": emb_k}
    nc = _get_nc()
    in_maps = _make_in_maps(inputs)
    res = run_bass_kernel_spmd(nc, in_maps, core_ids=list(range(NCORES)))
    out = np.empty((B, S, D), np.float32)
    for c in range(NCORES):
        b, qh = c // 2, c % 2
        out[b, qh * SQ:(qh + 1) * SQ] = res.results[c]["out"]
    return out
